# revision 1
# baseline (speedup 1.0000x reference)
# Trainium2 Bass kernel for single-head causal attention
#   q = x@Wq, k = x@Wk, v = x@Wv   (x [B,T,C], W* [C,H])
#   out = softmax(mask(q k^T / sqrt(C))) @ v
# B=512, T=142, C=512, H=64.  Data-parallel over B across 8 NeuronCores.
#
# Device-side layout strategy (per core, 64 batches = 9088 tokens):
#  - host feeds x^T  [4,128,9088]  (contraction dim C on partitions)
#  - qT = Wq-stationary matmuls -> psum [128,*] rows 0:64 (Wq zero-padded)
#  - k,v packed:  [Wk|Wv] stationary -> psum rows 0:64 = kT, 64:128 = vT
#  - scores weiT[s,t] = kT-stationary matmul; causal mask added via one
#    identity-stationary matmul accumulating a mask tile into PSUM
#  - exp on ScalarE (scale=C^-0.5 fused), result bf16 in SBUF
#  - v natural [s,h] via identity-matmul transpose of vT
#  - AV: exp-scores stationary, rhs = [v | ones] -> out [t, 65] where
#    col 64 = softmax denominator; division happens on host (glue).
# Groups of 3 batches; group PAIRS share one x DMA and one output DMA
# to keep the SP sequencer / HWDGE ring off the critical path.
import os

import numpy as np
import ml_dtypes

B, T, C, H = 512, 142, 512, 64
NCORES = 8
NB = B // NCORES            # 64 batches per core
NT = NB * T                 # 9088 tokens per core
GB = 3                      # batches per processing group
NG = (NB + GB - 1) // GB    # 22 groups (21 full + 1 single)
SCALE = float(C) ** -0.5
NEG = -1e30
TW = 65                     # out block width: H + 1 denominator column

_CACHE = {}


def _groups():
    return [(g * GB, min(GB, NB - g * GB)) for g in range(NG)]


def _build_nc():
    import concourse.bacc as bacc
    import concourse.mybir as mybir
    from concourse.tile import TileContext

    fp32 = mybir.dt.float32
    bf16 = mybir.dt.bfloat16
    Exp = mybir.ActivationFunctionType.Exp

    nc = bacc.Bacc(
        "TRN2",
        target_bir_lowering=False,
        debug=False,
        enable_asserts=False,
        num_devices=NCORES,
    )

    xt = nc.dram_tensor("xt", [4, 128, NT], bf16, kind="ExternalInput").ap()
    # all 8 weight chunks in one tensor: [Wq|0] chunks then [Wk|Wv] chunks
    wts = nc.dram_tensor("wts", [8, 128, 128], bf16, kind="ExternalInput").ap()
    # constants blob: cols 0:426 mask3, 426:468 mskt3 (rows 0:14),
    # 468:596 identity128, 596:660 idhi
    cst = nc.dram_tensor("cst", [128, 660], bf16, kind="ExternalInput").ap()
    om = nc.dram_tensor("om", [NG, 128, GB * TW], fp32, kind="ExternalOutput").ap()
    ot = nc.dram_tensor("ot", [NG, 14, GB * TW], fp32, kind="ExternalOutput").ap()

    GT = GB * T           # 426 token columns per full group
    TAIL0 = GT            # col offset of tail score blocks in psc
    groups = _groups()
    pairs = [(2 * p, min(2, NG - 2 * p)) for p in range((NG + 1) // 2)]

    with TileContext(nc) as tc:
        with (
            tc.tile_pool(name="const", bufs=1) as cpool,
            tc.tile_pool(name="xtp", bufs=2) as xpool,
            tc.tile_pool(name="work", bufs=3) as wpool,
            tc.tile_pool(name="psum", bufs=1, space="PSUM") as ppool,
        ):
            wts_sb = cpool.tile([128, 8 * 128], bf16)
            cst_sb = cpool.tile([128, 660], bf16)
            nc.sync.dma_start(
                out=wts_sb.rearrange("p (c w) -> p c w", c=8),
                in_=wts.rearrange("c p w -> p c w"))
            nc.sync.dma_start(out=cst_sb[:, :], in_=cst)

            def wq_c(c):
                return wts_sb[:, c * 128:(c + 1) * 128]

            def wkv_c(c):
                return wts_sb[:, 512 + c * 128:512 + (c + 1) * 128]

            msk3_sb = cst_sb[:, 0:426]
            mskt3_sb = cst_sb[0:14, 426:468]
            iden_sb = cst_sb[:, 468:596]
            idhi_sb = cst_sb[:, 596:660]

            for g0, np_ in pairs:
                pg = groups[g0:g0 + np_]
                gtp = sum(nb for _, nb in pg) * T
                t0 = pg[0][0] * T

                xt_t = xpool.tile([128, 4 * 2 * GT], bf16, tag="xt")
                if g0 == 0:
                    for c in range(4):
                        nc.sync.dma_start(
                            out=xt_t[:, c * gtp:(c + 1) * gtp],
                            in_=xt[c, :, t0:t0 + gtp],
                        )
                else:
                    nc.sync.dma_start(
                        out=xt_t[:, 0:4 * gtp].rearrange("p (c t) -> p c t", c=4),
                        in_=xt[:, :, t0:t0 + gtp].rearrange("c p t -> p c t"),
                    )

                o_sb = wpool.tile([128, 2 * GB * TW], fp32, tag="o")
                o2_sb = wpool.tile([14, 2 * GB * TW], fp32, tag="o2")

                for s, (b0, nb) in enumerate(pg):
                    gt = nb * T
                    off = (b0 * T) - t0          # token offset within pair tile

                    # ---- QKV projections ----
                    pq = ppool.tile([128, GT], fp32, tag="pq", bufs=2)
                    pkv = ppool.tile([128, GT], fp32, tag="pkv", bufs=2)
                    # kv first: the ACT kv-copy (scores' stationary operand)
                    # then overlaps the q matmuls on PE
                    for c in range(4):
                        rhs = xt_t[:, c * gtp + off:c * gtp + off + gt]
                        nc.tensor.matmul(
                            pkv[:, :gt], lhsT=wkv_c(c), rhs=rhs,
                            start=(c == 0), stop=(c == 3),
                        )
                    q_sb = wpool.tile([64, GT], bf16, tag="q")
                    kv_sb = wpool.tile([128, GT], bf16, tag="kv")
                    nc.scalar.copy(kv_sb[:, :gt], pkv[:, :gt])
                    for c in range(4):
                        rhs = xt_t[:, c * gtp + off:c * gtp + off + gt]
                        nc.tensor.matmul(
                            pq[:, :gt], lhsT=wq_c(c), rhs=rhs,
                            start=(c == 0), stop=(c == 3),
                        )
                    nc.vector.tensor_copy(q_sb[:, :gt], pq[0:64, :gt])

                    # ---- scores weiT[s,t] + causal mask ----
                    psc = ppool.tile([128, GT + GB * 14], fp32, tag="psc", bufs=2)
                    for j in range(nb):
                        cl = j * T
                        nc.tensor.matmul(
                            psc[:, cl:cl + T],
                            lhsT=kv_sb[0:64, cl:cl + 128],
                            rhs=q_sb[0:64, cl:cl + T],
                            start=True, stop=False,
                        )
                        nc.tensor.matmul(
                            psc[:, cl:cl + T],
                            lhsT=iden_sb,
                            rhs=msk3_sb[:, 0:T],
                            start=False, stop=True,
                        )
                        tco = TAIL0 + j * 14
                        nc.tensor.matmul(
                            psc[0:14, tco:tco + 14],
                            lhsT=kv_sb[0:64, cl + 128:cl + T],
                            rhs=q_sb[0:64, cl + 128:cl + T],
                            start=True, stop=False,
                        )
                        nc.tensor.matmul(
                            psc[0:14, tco:tco + 14],
                            lhsT=iden_sb[0:14, 0:14],
                            rhs=mskt3_sb[:, 0:14],
                            start=False, stop=True,
                        )

                    exp_sb = wpool.tile([128, GT + GB * 14], bf16, tag="exp")
                    nc.scalar.activation(
                        exp_sb[:, 0:gt], psc[:, 0:gt], Exp, scale=SCALE)
                    nc.scalar.activation(
                        exp_sb[0:14, TAIL0:TAIL0 + nb * 14],
                        psc[0:14, TAIL0:TAIL0 + nb * 14],
                        Exp, scale=SCALE,
                    )

                    # ---- v natural via identity-matmul transpose ----
                    pvt = ppool.tile([128, GB * 128], fp32, tag="pvt")
                    for j in range(nb):
                        cl = j * T
                        nc.tensor.matmul(
                            pvt[:, j * 64:(j + 1) * 64],
                            lhsT=kv_sb[64:128, cl:cl + 128],
                            rhs=idhi_sb[64:128, :],
                            start=True, stop=True,
                        )
                        nc.tensor.matmul(
                            pvt[0:14, GB * 64 + j * 64:GB * 64 + (j + 1) * 64],
                            lhsT=kv_sb[64:128, cl + 128:cl + T],
                            rhs=idhi_sb[64:128, :],
                            start=True, stop=True,
                        )
                    vex_sb = wpool.tile([128, GB * TW], bf16, tag="vex")
                    vext_sb = wpool.tile([14, GB * TW], bf16, tag="vext")
                    nc.vector.tensor_copy(
                        vex_sb.rearrange("p (b h) -> p b h", h=TW)[:, 0:nb, 0:64],
                        pvt[:, 0:nb * 64].rearrange("p (b h) -> p b h", h=64),
                    )
                    nc.vector.tensor_copy(
                        vext_sb.rearrange("p (b h) -> p b h", h=TW)[:, 0:nb, 0:64],
                        pvt[0:14, GB * 64:GB * 64 + nb * 64].rearrange(
                            "p (b h) -> p b h", h=64),
                    )
                    nc.vector.memset(
                        vex_sb.rearrange("p (b h) -> p b h", h=TW)[:, 0:nb, 64:65],
                        1.0)
                    nc.vector.memset(
                        vext_sb.rearrange("p (b h) -> p b h", h=TW)[:, 0:nb, 64:65],
                        1.0)

                    # ---- AV: out[t,0:64] = sum_s P^T[s,t] v[s,:], col64=denom ----
                    pout = ppool.tile([128, 2 * GB * TW], fp32, tag="pout")
                    TL = GB * TW
                    for j in range(nb):
                        cl = j * T
                        nc.tensor.matmul(
                            pout[:, j * TW:(j + 1) * TW],
                            lhsT=exp_sb[:, cl:cl + 128],
                            rhs=vex_sb[:, j * TW:(j + 1) * TW],
                            start=True, stop=True,
                        )
                        nc.tensor.matmul(
                            pout[0:14, TL + j * TW:TL + (j + 1) * TW],
                            lhsT=exp_sb[:, cl + 128:cl + T],
                            rhs=vex_sb[:, j * TW:(j + 1) * TW],
                            start=True, stop=False,
                        )
                        nc.tensor.matmul(
                            pout[0:14, TL + j * TW:TL + (j + 1) * TW],
                            lhsT=exp_sb[0:14, TAIL0 + j * 14:TAIL0 + (j + 1) * 14],
                            rhs=vext_sb[0:14, j * TW:(j + 1) * TW],
                            start=False, stop=True,
                        )

                    oc = s * GB * TW
                    nc.scalar.copy(
                        o_sb[:, oc:oc + nb * TW], pout[:, 0:nb * TW])
                    nc.vector.tensor_copy(
                        o2_sb[0:14, oc:oc + nb * TW],
                        pout[0:14, TL:TL + nb * TW])

                # ---- batched output stores (one per pair per tensor) ----
                last_nb = pg[-1][1]
                if np_ == 2 and last_nb == GB:
                    nc.gpsimd.dma_start(
                        out=om[g0:g0 + 2].rearrange("g p c -> p g c"),
                        in_=o_sb.rearrange("p (g c) -> p g c", g=2),
                    )
                    nc.gpsimd.dma_start(
                        out=ot[g0:g0 + 2].rearrange("g p c -> p g c"),
                        in_=o2_sb.rearrange("p (g c) -> p g c", g=2),
                    )
                else:
                    for s, (b0, nb) in enumerate(pg):
                        oc = s * GB * TW
                        nc.gpsimd.dma_start(
                            out=om[g0 + s, :, 0:nb * TW],
                            in_=o_sb[:, oc:oc + nb * TW])
                        nc.gpsimd.dma_start(
                            out=ot[g0 + s, :, 0:nb * TW],
                            in_=o2_sb[0:14, oc:oc + nb * TW])

    nc.compile()
    return nc


def _prep_shared(Wq, Wk, Wv):
    bf16 = ml_dtypes.bfloat16
    wq_pad = np.concatenate([Wq, np.zeros((C, H), np.float32)], axis=1)
    wkv = np.concatenate([Wk, Wv], axis=1)
    wts_np = np.concatenate(
        [
            np.ascontiguousarray(wq_pad.reshape(4, 128, 128)),
            np.ascontiguousarray(wkv.reshape(4, 128, 128)),
        ],
        axis=0,
    ).astype(bf16)

    s = np.arange(128)[:, None]
    t = np.arange(T)[None, :]
    msk = np.where(s <= t, 0.0, NEG).astype(np.float32)
    i = np.arange(14)[:, None]
    j = np.arange(14)[None, :]
    mskt = np.where(i <= j, 0.0, NEG).astype(np.float32)
    idhi = np.zeros((128, 64), np.float32)
    idhi[64 + np.arange(64), np.arange(64)] = 1.0

    cst = np.zeros((128, 660), np.float32)
    cst[:, 0:426] = np.tile(msk, (1, 3))
    cst[0:14, 426:468] = np.tile(mskt, (1, 3))
    cst[:, 468:596] = np.eye(128, dtype=np.float32)
    cst[:, 596:660] = idhi
    return dict(wts=wts_np, cst=cst.astype(bf16))


def _prep_core_xt(x_core):
    # x_core [NB, T, C] fp32 -> [4, 128, NT] bf16 (x^T, C on partitions)
    xt = x_core.reshape(NT, C).T            # [C, NT] view
    xt = np.ascontiguousarray(xt).reshape(4, 128, NT)
    return xt.astype(ml_dtypes.bfloat16)


def _assemble_core(om_np, ot_np):
    # om [NG, 128, GB*TW], ot [NG, 14, GB*TW] -> [NB, T, H] normalized
    bm = om_np.reshape(NG, 128, GB, TW).transpose(0, 2, 1, 3).reshape(NG * GB, 128, TW)
    bt = ot_np.reshape(NG, 14, GB, TW).transpose(0, 2, 1, 3).reshape(NG * GB, 14, TW)
    bm = bm[:NB].astype(np.float32)
    bt = bt[:NB].astype(np.float32)
    full = np.concatenate([bm, bt], axis=1)         # [NB, 142, TW]
    return full[:, :, 0:H] / full[:, :, H:H + 1]


def kernel(**inputs):
    x = np.asarray(inputs["x"], dtype=np.float32)
    Wq = np.asarray(inputs["Wq"], dtype=np.float32)
    Wk = np.asarray(inputs["Wk"], dtype=np.float32)
    Wv = np.asarray(inputs["Wv"], dtype=np.float32)

    from concourse.bass_utils import run_bass_kernel_spmd

    if "nc" not in _CACHE:
        _CACHE["nc"] = _build_nc()
    nc = _CACHE["nc"]

    shared = _prep_shared(Wq, Wk, Wv)
    in_maps = []
    for core in range(NCORES):
        m = dict(shared)
        m["xt"] = _prep_core_xt(x[core * NB:(core + 1) * NB])
        in_maps.append(m)

    trace = bool(int(os.environ.get("TRN_KERNEL_TRACE", "0")))
    res = run_bass_kernel_spmd(
        nc, in_maps, core_ids=list(range(NCORES)), trace=trace,
    )
    _CACHE["last_result"] = res

    outs = []
    for core in range(NCORES):
        r = res.results[core]
        outs.append(_assemble_core(np.asarray(r["om"]), np.asarray(r["ot"])))
    return np.concatenate(outs, axis=0).astype(np.float32)



# revision 23
# speedup vs baseline: 1.1396x; 1.1396x over previous
# Trainium2 Bass kernel for single-head causal attention
#   q = x@Wq, k = x@Wk, v = x@Wv   (x [B,T,C], W* [C,H])
#   out = softmax(mask(q k^T / sqrt(C))) @ v
# B=512, T=142, C=512, H=64.  Data-parallel over B across 8 NeuronCores.
#
# Strategy (per core, 64 batches, groups of 3, group pairs share DMAs):
#  - x shipped as fp8e4 pair (x8, 16*(x-x8)) => bf16-parity operand at
#    1 B/elem per tensor; projections run as fp8 DoubleRow matmuls
#    (cost model: 0.5 cycles/out-col, contraction 256/instr) with
#    weights pre-scaled by 64 (and a /16 copy for the residual term)
#    to dodge e4m3's subnormal range.  k,v get the full (x8+dx) parity;
#    q uses x8 only (rel err ~9e-3 vs 2e-2 gate).
#  - scores/AV stay bf16; causal mask is ONE fp8e5 DoubleRow matmul
#    per group (diag=4096 stationary x mask{0,-1024} = -4.2e6 in psum).
#  - psc layout per batch: [142 main | 14 tail]; one exp (ScalarE) per
#    group covers main+tail; tail rows 14:128 are mask-poisoned so
#    exp()=0 there (feeds the AV tail matmul safely).
#  - v natural via identity matmul transpose; vex/vext merged in one
#    [128,390] tile per group with static ones columns (denominator).
#  - out cols [main 195 | tail 195] staged bf16, one DMA per pair.
#  - engines: PE matmuls; ACT kv-copy + exp; DVE q-copy + vex-copy;
#    Pool out-copy; SP all DMAs (x prefetched 2 pairs deep).
import os

import numpy as np
import ml_dtypes

B, T, C, H = 512, 142, 512, 64
NCORES = 8
NB = B // NCORES            # 64 batches per core
NT = NB * T                 # 9088 tokens per core
GB = 3                      # batches per processing group
NG = (NB + GB - 1) // GB    # 22 groups (21 full + 1 single)
NPAIR = (NG + 1) // 2       # 11 pairs
SCALE = float(C) ** -0.5
WS = 64.0                   # weight pre-scale (fp8 subnormal dodge)
DS = 16.0                   # dx pre-scale
TW = 65                     # out block width: H + 1 denominator column
GT = GB * T                 # 426 tokens per full group
GTP = 864                   # padded chunk stride in x sbuf tile (%16==0)
PSC = GB * (T + 14)         # 468 score cols per group (156 per batch)
Q_XTERMS = 4                # q-pass matmuls: 2=W8 only, 4=+dW8, 6=+dx term

_CACHE = {}

e4np = ml_dtypes.float8_e4m3
e5np = ml_dtypes.float8_e5m2
bfnp = ml_dtypes.bfloat16


def _groups():
    return [(g * GB, min(GB, NB - g * GB)) for g in range(NG)]


def _build_nc():
    import concourse.bacc as bacc
    import concourse.mybir as mybir
    from concourse.tile import TileContext

    fp32 = mybir.dt.float32
    bf16 = mybir.dt.bfloat16
    f8e4 = mybir.dt.float8e4
    f8e5 = mybir.dt.float8e5
    DR = mybir.MatmulPerfMode.DoubleRow
    Exp = mybir.ActivationFunctionType.Exp

    nc = bacc.Bacc(
        "TRN2",
        target_bir_lowering=False,
        debug=False,
        enable_asserts=False,
        num_devices=NCORES,
    )

    # x8 / dx8s chunks: [hl, c, 128, NT] fp8e4
    xt8 = nc.dram_tensor("xt8", [2, 4, 128, NT], f8e4, kind="ExternalInput").ap()
    # weight blocks fp8e4 (see _prep_shared): kvW8 cc0|cc1, kvdW8, kvW8d,
    # qW8, qdW8, qW8d
    wts = nc.dram_tensor("wts", [128, 2304], f8e4, kind="ExternalInput").ap()
    # e5m2 consts: iden_dr [p,2,128] then mask_dr [p,2,480]
    cst8 = nc.dram_tensor("cst8", [128, 256 + 960], f8e5, kind="ExternalInput").ap()
    # bf16 consts: idhi
    cstb = nc.dram_tensor("cstb", [128, 64], bf16, kind="ExternalInput").ap()
    om = nc.dram_tensor("om", [NPAIR, 128, 2 * GB * TW], bf16,
                        kind="ExternalOutput").ap()
    ot = nc.dram_tensor("ot", [NPAIR, 14, 2 * GB * TW], bf16,
                        kind="ExternalOutput").ap()

    groups = _groups()
    pairs = [(2 * p, min(2, NG - 2 * p)) for p in range(NPAIR)]

    with TileContext(nc) as tc:
        with (
            tc.tile_pool(name="const", bufs=1) as cpool,
            tc.tile_pool(name="xtp", bufs=2) as xpool,
            tc.tile_pool(name="work", bufs=3) as wpool,
            tc.tile_pool(name="psum", bufs=1, space="PSUM") as ppool,
        ):
            wts_sb = cpool.tile([128, 2304], f8e4)
            cst8_sb = cpool.tile([128, 1216], f8e5)
            cstb_sb = cpool.tile([128, 64], bf16)
            vex_all = cpool.tile([128, 2 * 2 * GB * TW], bf16)  # 2 persistent bufs
            nc.sync.dma_start(out=wts_sb[:, :], in_=wts)
            nc.sync.dma_start(out=cst8_sb[:, :], in_=cst8)
            nc.sync.dma_start(out=cstb_sb[:, :], in_=cstb)
            # static ones columns (softmax denominator) in both vex buffers
            nc.vector.memset(
                vex_all.rearrange("p (k w) -> p k w", w=TW)[:, :, 64:65], 1.0)

            def kv_w8(cc):
                return wts_sb[:, cc * 256:(cc + 1) * 256].rearrange(
                    "p (i m) -> p i m", i=2)

            def kv_dw8(cc):
                return wts_sb[:, 512 + cc * 256:512 + (cc + 1) * 256].rearrange(
                    "p (i m) -> p i m", i=2)

            def kv_w8d(cc):
                return wts_sb[:, 1024 + cc * 256:1024 + (cc + 1) * 256].rearrange(
                    "p (i m) -> p i m", i=2)

            def q_w8(cc):
                return wts_sb[:, 1536 + cc * 128:1536 + (cc + 1) * 128].rearrange(
                    "p (i m) -> p i m", i=2)

            def q_dw8(cc):
                return wts_sb[:, 1792 + cc * 128:1792 + (cc + 1) * 128].rearrange(
                    "p (i m) -> p i m", i=2)

            def q_w8d(cc):
                return wts_sb[:, 2048 + cc * 128:2048 + (cc + 1) * 128].rearrange(
                    "p (i m) -> p i m", i=2)

            iden_dr = cst8_sb[:, 0:256].rearrange("p (i m) -> p i m", i=2)
            mask_dr = cst8_sb[:, 256:1216].rearrange("p (i m) -> p i m", i=2)
            idhi_sb = cstb_sb

            xt_tiles = {}

            def emit_x_dma(pi):
                g0, np_ = pairs[pi]
                t0 = groups[g0][0] * T
                gtp = sum(groups[g0 + s][1] for s in range(np_)) * T
                xt_t = xpool.tile([128, 8 * GTP], f8e4, tag="xt")
                nc.gpsimd.dma_start(
                    out=xt_t.rearrange(
                        "p (hl c t) -> p hl c t", hl=2, c=4)[:, :, :, 0:gtp],
                    in_=xt8[:, :, :, t0:t0 + gtp].rearrange("hl c p t -> p hl c t"),
                )
                xt_tiles[pi] = (xt_t, gtp)

            emit_x_dma(0)
            emit_x_dma(1)

            for pi, (g0, np_) in enumerate(pairs):
                xt_t, gtp = xt_tiles.pop(pi)
                xv = xt_t.rearrange("p (g t) -> p g t", g=8)  # g = hl*4 + c
                t0 = groups[g0][0] * T

                o_sb = wpool.tile([128, 2 * 2 * GB * TW], bf16, tag="o")

                for s in range(np_):
                    b0, nb = groups[g0 + s]
                    gt = nb * T
                    off = b0 * T - t0
                    scn = nb * (T + 14)

                    def x8v(cc, lo=off, n=gt):
                        return xv[:, 2 * cc:2 * cc + 2, lo:lo + n]

                    def dx8v(cc, lo=off, n=gt):
                        return xv[:, 4 + 2 * cc:4 + 2 * cc + 2, lo:lo + n]

                    # ---- KV projection: (x8+dx) (W8+dW8), scaled by 64 ----
                    pkv = ppool.tile([128, GT], fp32, tag="pkv", bufs=2)
                    seq = [(kv_w8(0), x8v(0)), (kv_w8(1), x8v(1)),
                           (kv_dw8(0), x8v(0)), (kv_dw8(1), x8v(1)),
                           (kv_w8d(0), dx8v(0)), (kv_w8d(1), dx8v(1))]
                    for i, (lhs, rhs) in enumerate(seq):
                        nc.tensor.matmul(
                            pkv[:, 0:gt], lhsT=lhs, rhs=rhs,
                            start=(i == 0), stop=(i == len(seq) - 1),
                            perf_mode=DR,
                        )
                    kv_sb = wpool.tile([128, GT], bf16, tag="kv")
                    nc.scalar.copy(kv_sb[:, 0:gt], pkv[:, 0:gt])

                    # ---- Q projection ----
                    pq = ppool.tile([64, GT], fp32, tag="pq", bufs=2)
                    seq = [(q_w8(0), x8v(0)), (q_w8(1), x8v(1))]
                    if Q_XTERMS >= 4:
                        seq += [(q_dw8(0), x8v(0)), (q_dw8(1), x8v(1))]
                    if Q_XTERMS >= 6:
                        seq += [(q_w8d(0), dx8v(0)), (q_w8d(1), dx8v(1))]
                    for i, (lhs, rhs) in enumerate(seq):
                        nc.tensor.matmul(
                            pq[:, 0:gt], lhsT=lhs, rhs=rhs,
                            start=(i == 0), stop=(i == len(seq) - 1),
                            perf_mode=DR,
                        )
                    q_sb = wpool.tile([64, GT], bf16, tag="q")
                    nc.vector.tensor_copy(q_sb[:, 0:gt], pq[:, 0:gt])

                    # ---- v natural via identity-matmul transpose ----
                    pvt = ppool.tile([128, 2 * GB * 64], fp32, tag="pvt")
                    for j in range(nb):
                        cl = j * T
                        nc.tensor.matmul(
                            pvt[:, j * 64:(j + 1) * 64],
                            lhsT=kv_sb[64:128, cl:cl + 128],
                            rhs=idhi_sb[64:128, :],
                            start=True, stop=True,
                        )
                        nc.tensor.matmul(
                            pvt[0:14, GB * 64 + j * 64:GB * 64 + (j + 1) * 64],
                            lhsT=kv_sb[64:128, cl + 128:cl + T],
                            rhs=idhi_sb[64:128, :],
                            start=True, stop=True,
                        )
                    vex = vex_all[:, ((g0 + s) % 2) * 390:((g0 + s) % 2) * 390 + 390]
                    vex_v = vex.rearrange("p (i j w) -> p i j w", i=2, w=TW)
                    pvt_v = pvt.rearrange("p (i j t) -> p i j t", i=2, j=GB)
                    nc.vector.tensor_copy(
                        vex_v[:, 0:1, 0:nb, 0:64], pvt_v[:, 0:1, 0:nb, :])
                    nc.scalar.copy(
                        vex_v[0:14, 1:2, 0:nb, 0:64], pvt_v[0:14, 1:2, 0:nb, :])

                    # ---- scores [142 main | 14 tail] per batch + mask ----
                    # Mask goes FIRST with start=True: one DR matmul writes
                    # the whole psc region (incl. the -4.2e6 poison on tail
                    # partitions 14:128), then the score matmuls accumulate
                    # onto it with start=False.  This keeps every PSUM byte's
                    # has_written state uniform per instruction.
                    psc = ppool.tile([128, PSC], fp32, tag="psc", bufs=2)
                    nc.tensor.matmul(
                        psc[:, 0:scn],
                        lhsT=iden_dr,
                        rhs=mask_dr[:, :, 0:scn],
                        start=True, stop=False,
                        perf_mode=DR,
                        skip_group_check=True,
                    )
                    for j in range(nb):
                        cl = j * T
                        sc = j * (T + 14)
                        nc.tensor.matmul(
                            psc[:, sc:sc + T],
                            lhsT=kv_sb[0:64, cl:cl + 128],
                            rhs=q_sb[0:64, cl:cl + T],
                            start=False, stop=False,
                            skip_group_check=True,
                        )
                        nc.tensor.matmul(
                            psc[0:14, sc + T:sc + T + 14],
                            lhsT=kv_sb[0:64, cl + 128:cl + T],
                            rhs=q_sb[0:64, cl + 128:cl + T],
                            start=False, stop=(j == nb - 1),
                            skip_group_check=True,
                        )
                    exp_sb = wpool.tile([128, PSC], bf16, tag="exp")
                    nc.scalar.activation(
                        exp_sb[:, 0:scn], psc[:, 0:scn], Exp,
                        scale=SCALE / (WS * WS))

                    # ---- AV: out[t,0:64] + denominator col 64 ----
                    pout = ppool.tile([128, 2 * GB * TW], fp32, tag="pout")
                    TL = GB * TW
                    for j in range(nb):
                        sc = j * (T + 14)
                        nc.tensor.matmul(
                            pout[:, j * TW:(j + 1) * TW],
                            lhsT=exp_sb[:, sc:sc + 128],
                            rhs=vex[:, j * TW:(j + 1) * TW],
                            start=True, stop=True,
                        )
                        nc.tensor.matmul(
                            pout[0:14, TL + j * TW:TL + (j + 1) * TW],
                            lhsT=exp_sb[:, sc + 128:sc + T],
                            rhs=vex[:, j * TW:(j + 1) * TW],
                            start=True, stop=False,
                        )
                        nc.tensor.matmul(
                            pout[0:14, TL + j * TW:TL + (j + 1) * TW],
                            lhsT=exp_sb[0:14, sc + T:sc + T + 14],
                            rhs=vex[0:14, TL + j * TW:TL + (j + 1) * TW],
                            start=False, stop=True,
                        )
                    nc.vector.tensor_copy(
                        o_sb[:, s * 195:s * 195 + nb * TW], pout[:, 0:nb * TW])
                    nc.vector.tensor_copy(
                        o_sb[0:14, 390 + s * 195:390 + s * 195 + nb * TW],
                        pout[0:14, 195:195 + nb * TW])

                ocols = 195 * (np_ - 1) + groups[g0 + np_ - 1][1] * TW
                nc.sync.dma_start(
                    out=om[pi][:, 0:ocols], in_=o_sb[:, 0:ocols])
                nc.sync.dma_start(
                    out=ot[pi][:, 0:ocols], in_=o_sb[0:14, 390:390 + ocols])
                if pi + 2 < NPAIR:
                    emit_x_dma(pi + 2)

    nc.compile()
    return nc


def _prep_shared(Wq, Wk, Wv):
    def q8(a):
        return a.astype(e4np).astype(np.float32)

    def blocks(W8, m):
        # cc blocks of [p, i, m] = W8[(2cc+i)*128+p, m], flattened to cols
        r = W8.reshape(4, 128, m)
        return [np.stack([r[2 * cc], r[2 * cc + 1]], axis=1).reshape(128, 2 * m)
                for cc in range(2)]

    Wkv = np.concatenate([Wk, Wv], axis=1) * WS          # [512, 128]
    kvW8 = q8(Wkv)
    kvdW8 = q8(Wkv - kvW8)
    kvW8d = q8(Wkv / DS)
    Wqs = Wq * WS                                        # [512, 64]
    qW8 = q8(Wqs)
    qdW8 = q8(Wqs - qW8)
    qW8d = q8(Wqs / DS)

    cols = (blocks(kvW8, 128) + blocks(kvdW8, 128) + blocks(kvW8d, 128)
            + blocks(qW8, 64) + blocks(qdW8, 64) + blocks(qW8d, 64))
    wts = np.concatenate(cols, axis=1).astype(e4np)      # [128, 2816]

    iden = np.zeros((128, 2, 128), np.float32)
    iden[np.arange(128), 0, np.arange(128)] = 4096.0
    maskv = np.zeros((128, 2, 480), np.float32)
    s = np.arange(128)[:, None]
    t = np.arange(T)[None, :]
    mmain = np.where(s <= t, 0.0, -1024.0)               # [128, 142]
    i = np.arange(128)[:, None]
    j = np.arange(14)[None, :]
    mtail = np.where((i < 14) & (i <= j), 0.0, -1024.0)  # [128, 14]
    per_b = np.concatenate([mmain, mtail], axis=1)       # [128, 156]
    maskv[:, 0, 0:PSC] = np.tile(per_b, (1, GB))
    cst8 = np.concatenate(
        [iden.reshape(128, 256), maskv.reshape(128, 960)], axis=1).astype(e5np)

    idhi = np.zeros((128, 64), np.float32)
    idhi[64 + np.arange(64), np.arange(64)] = 1.0
    return dict(wts=wts, cst8=cst8, cstb=idhi.astype(bfnp))


def _prep_core_xt(x_core):
    # x_core [NB, T, C] fp32 -> [2, 4, 128, NT] fp8e4: (x8, 16*(x-x8)) chunks
    xt = np.ascontiguousarray(x_core.reshape(NT, C).T)   # [C, NT]
    x8 = xt.astype(e4np)
    dx8s = (DS * (xt - x8.astype(np.float32))).astype(e4np)
    return np.stack(
        [x8.reshape(4, 128, NT), dx8s.reshape(4, 128, NT)]).astype(e4np)


def _assemble_core(om_np, ot_np):
    # om [NPAIR, 128, 2*GB*TW], ot [NPAIR, 14, 2*GB*TW] -> [NB, T, H], /WS
    o = om_np.astype(np.float32).reshape(NPAIR, 128, 2, GB, TW)
    o2 = ot_np.astype(np.float32).reshape(NPAIR, 14, 2, GB, TW)
    out = np.empty((NB, T, TW), np.float32)
    for pi in range(NPAIR):
        for g in range(2):
            gi = 2 * pi + g
            if gi >= NG:
                break
            b0, nb = gi * GB, min(GB, NB - gi * GB)
            for j in range(nb):
                out[b0 + j, 0:128, :] = o[pi, :, g, j, :]
                out[b0 + j, 128:142, :] = o2[pi, :, g, j, :]
    return (out[:, :, 0:H] / out[:, :, H:H + 1]) / WS


def kernel(**inputs):
    x = np.asarray(inputs["x"], dtype=np.float32)
    Wq = np.asarray(inputs["Wq"], dtype=np.float32)
    Wk = np.asarray(inputs["Wk"], dtype=np.float32)
    Wv = np.asarray(inputs["Wv"], dtype=np.float32)

    from concourse.bass_utils import run_bass_kernel_spmd

    if "nc" not in _CACHE:
        _CACHE["nc"] = _build_nc()
    nc = _CACHE["nc"]

    shared = _prep_shared(Wq, Wk, Wv)
    in_maps = []
    for core in range(NCORES):
        m = dict(shared)
        m["xt8"] = _prep_core_xt(x[core * NB:(core + 1) * NB])
        in_maps.append(m)

    trace = bool(int(os.environ.get("TRN_KERNEL_TRACE", "0")))
    res = run_bass_kernel_spmd(
        nc, in_maps, core_ids=list(range(NCORES)), trace=trace,
    )
    _CACHE["last_result"] = res

    outs = []
    for core in range(NCORES):
        r = res.results[core]
        outs.append(_assemble_core(np.asarray(r["om"]), np.asarray(r["ot"])))
    return np.concatenate(outs, axis=0).astype(np.float32)
